# revision 7
# baseline (speedup 1.0000x reference)
"""VQ-VAE forward kernel for Trainium2 (Bass/Tile), data-parallel over 8 NeuronCores.

Pipeline per core (rows sharded 8-way, ROWS=16384 rows/core, blocks of R=512 rows):
  encoder    y1T = relu(W1.T @ xT + b1)      [512, R]  (activations kept transposed)
             zT  = W2.T @ y1T + b2           [256, R]
  vq         dots[i] = z @ cbT  (row-major, 128-row chunks), negd = dots - ||c||^2/2
             argmin dist == argmax negd  ->  max / max_index  (top-8 DVE ops)
  gather     one-hot(idx) matmul against codebook -> zqT [256, R]
  decoder    hT = relu(W3.T @ zqT + b3);  recon[i] = hT.T @ W4 + b4 (row-major out)
  loss       sum(z^2) - 2*sum(max0) partials, reduced on host.

Matmul dtype is float32r (full PE rate with fp32 storage) by default, selectable
per stage group below.
"""

import numpy as np

import concourse.bass as bass
from concourse import bacc
import concourse.tile as tile
import concourse.mybir as mybir
from concourse.bass_utils import run_bass_kernel_spmd
from concourse.masks import make_identity
from concourse import library_config

N, D_IN, D_HID, D_CODE, K = 131072, 768, 512, 256, 512
NCORES = 8
ROWS = N // NCORES          # rows per core
RBLK = 512                  # rows per block
F32 = mybir.dt.float32
U32 = mybir.dt.uint32
I32 = mybir.dt.int32
AF = mybir.ActivationFunctionType
ALU = mybir.AluOpType


def _mm(nc, out, lhsT, rhs, dt, start, stop):
    nc.tensor.matmul(out, lhsT, rhs, start=start, stop=stop)


def build_nc(rows=ROWS, enc_dt=F32, gat_dt=mybir.dt.float32r,
             dec_dt=mybir.dt.float32r):
    nblk = rows // RBLK
    nc = bacc.Bacc("TRN2", target_bir_lowering=False, debug=False)

    # inputs (per-core shard of xT; weights replicated)
    xT = nc.dram_tensor("xT", [D_IN, rows], enc_dt, kind="ExternalInput")
    W1 = nc.dram_tensor("W1", [D_IN, D_HID], enc_dt, kind="ExternalInput")
    W2 = nc.dram_tensor("W2", [D_HID, D_CODE], enc_dt, kind="ExternalInput")
    CB = nc.dram_tensor("CB", [K, D_CODE], gat_dt, kind="ExternalInput")
    CBT = nc.dram_tensor("CBT", [D_CODE, K], enc_dt, kind="ExternalInput")
    C2H = nc.dram_tensor("C2H", [K], F32, kind="ExternalInput")
    W3 = nc.dram_tensor("W3", [D_CODE, D_HID], dec_dt, kind="ExternalInput")
    W4 = nc.dram_tensor("W4", [D_HID, D_IN], dec_dt, kind="ExternalInput")
    B1 = nc.dram_tensor("B1", [128, 4], F32, kind="ExternalInput")   # b1[128c+p] at [p,c]
    B2 = nc.dram_tensor("B2", [128, 2], F32, kind="ExternalInput")
    B3 = nc.dram_tensor("B3", [128, 4], F32, kind="ExternalInput")
    B4 = nc.dram_tensor("B4", [D_IN], F32, kind="ExternalInput")
    IOTA = nc.dram_tensor("IOTA", [128, 4], F32, kind="ExternalInput")  # 128c+p at [p,c]

    recon = nc.dram_tensor("recon", [rows, D_IN], F32, kind="ExternalOutput")
    idx_o = nc.dram_tensor("idx", [rows], I32, kind="ExternalOutput")
    lz2_o = nc.dram_tensor("loss_z2", [128, 2 * nblk], F32, kind="ExternalOutput")
    lmx_o = nc.dram_tensor("loss_mx", [128, 4 * nblk], F32, kind="ExternalOutput")

    with tile.TileContext(nc) as tc, tc.tile_pool(name="consts", bufs=1) as consts, \
            tc.tile_pool(name="work", bufs=2) as work, \
            tc.tile_pool(name="small", bufs=4) as small, \
            tc.tile_pool(name="rout", bufs=4) as rout, \
            tc.tile_pool(name="psum", bufs=7, space="PSUM") as pp:
        # ---- constants into SBUF ----
        w1s = consts.tile([128, 6, D_HID], enc_dt)
        nc.sync.dma_start(w1s, W1.ap().rearrange("(c p) n -> p c n", p=128))
        w2s = consts.tile([128, 4, D_CODE], enc_dt)
        nc.sync.dma_start(w2s, W2.ap().rearrange("(c p) n -> p c n", p=128))
        cbs = consts.tile([128, 4, D_CODE], gat_dt)
        nc.sync.dma_start(cbs, CB.ap().rearrange("(c p) n -> p c n", p=128))
        cbts = consts.tile([128, 2, K], enc_dt)
        nc.sync.dma_start(cbts, CBT.ap().rearrange("(c p) n -> p c n", p=128))
        w3s = consts.tile([128, 2, D_HID], dec_dt)
        nc.sync.dma_start(w3s, W3.ap().rearrange("(c p) n -> p c n", p=128))
        w4s = consts.tile([128, 4, D_IN], dec_dt)
        nc.sync.dma_start(w4s, W4.ap().rearrange("(c p) n -> p c n", p=128))
        c2hb = consts.tile([128, K], F32)
        nc.sync.dma_start(c2hb, bass.AP(tensor=C2H, offset=0, ap=[[0, 128], [1, K]]))
        b4b = consts.tile([128, D_IN], F32)
        nc.sync.dma_start(b4b, bass.AP(tensor=B4, offset=0, ap=[[0, 128], [1, D_IN]]))
        b1c = consts.tile([128, 4], F32)
        nc.sync.dma_start(b1c, B1.ap())
        b2c = consts.tile([128, 2], F32)
        nc.sync.dma_start(b2c, B2.ap())
        b3c = consts.tile([128, 4], F32)
        nc.sync.dma_start(b3c, B3.ap())
        iotac = consts.tile([128, 4], F32)
        nc.sync.dma_start(iotac, IOTA.ap())
        ident = consts.tile([128, 128], F32)
        make_identity(nc, ident[:])
        nc.gpsimd.load_library(library_config.mlp)

        accz2 = consts.tile([128, 2 * nblk], F32)
        accmx = consts.tile([128, 4 * nblk], F32)

        xT_r = xT.ap().rearrange("(c p) r -> p c r", p=128)

        for b in range(nblk):
            r0 = b * RBLK
            xt = work.tile([128, 6, RBLK], enc_dt, tag="xt")
            nc.sync.dma_start(xt, xT_r[:, :, r0:r0 + RBLK])

            # ---- encoder layer 1: y1T[h, r] ----
            y1sb = work.tile([128, 4, RBLK], enc_dt, tag="y1")
            for m in range(4):
                ps = pp.tile([128, RBLK], F32, tag="ps")
                for k in range(6):
                    _mm(nc, ps, w1s[:, k, m * 128:(m + 1) * 128], xt[:, k, :],
                        enc_dt, k == 0, k == 5)
                nc.scalar.activation(y1sb[:, m, :], ps, AF.Relu, bias=b1c[:, m:m + 1])

            # ---- encoder layer 2: zT[c, r] ----
            zsb = work.tile([128, 2, RBLK], enc_dt, tag="z")
            for m in range(2):
                ps = pp.tile([128, RBLK], F32, tag="ps")
                for k in range(4):
                    _mm(nc, ps, w2s[:, k, m * 128:(m + 1) * 128], y1sb[:, k, :],
                        enc_dt, k == 0, k == 3)
                nc.scalar.activation(zsb[:, m, :], ps, AF.Identity, bias=b2c[:, m:m + 1])
                zsq = work.tile([128, RBLK], F32, tag="zsq")
                nc.scalar.activation(zsq, zsb[:, m, :], AF.Square,
                                     accum_out=accz2[:, 2 * b + m:2 * b + m + 1])

            # ---- distances + argmin per 128-row chunk ----
            ixf = small.tile([128, 4], F32, tag="ixf")
            for i in range(4):
                ps = pp.tile([128, K], F32, tag="ps")
                _mm(nc, ps, zsb[:, 0, i * 128:(i + 1) * 128], cbts[:, 0, :],
                    enc_dt, True, False)
                _mm(nc, ps, zsb[:, 1, i * 128:(i + 1) * 128], cbts[:, 1, :],
                    enc_dt, False, True)
                negd = work.tile([128, K], F32, tag="negd")
                nc.vector.scalar_tensor_tensor(negd, in0=ps, scalar=1.0, in1=c2hb,
                                               op0=ALU.mult, op1=ALU.subtract)
                mx = small.tile([128, 8], F32, tag="mx")
                nc.vector.max(mx, negd)
                ix = small.tile([128, 8], U32, tag="ix")
                nc.vector.max_index(ix, mx, negd)
                nc.scalar.copy(accmx[:, 4 * b + i:4 * b + i + 1], mx[:, 0:1])
                nc.gpsimd.tensor_copy(ixf[:, i:i + 1], ix[:, 0:1])

            # ---- transpose indices to [1, RBLK], write out, broadcast for one-hot ----
            tps = pp.tile([1, RBLK], F32, tag="ps")
            for i in range(4):
                nc.tensor.matmul(tps[:, i * 128:(i + 1) * 128], ixf[:, i:i + 1],
                                 ident, is_transpose=True, start=(i == 0), stop=(i == 3))
            ixT1 = small.tile([1, RBLK], F32, tag="ixT")
            nc.vector.tensor_copy(ixT1, tps)
            ixi = small.tile([1, RBLK], I32, tag="ixi")
            nc.vector.tensor_copy(ixi, ixT1)
            nc.sync.dma_start(idx_o.ap()[r0:r0 + RBLK].unsqueeze(0), ixi)
            ixb = work.tile([128, RBLK], F32, tag="ixb")
            nc.gpsimd.partition_broadcast(ixb, ixT1, channels=128)

            # ---- one-hot + gather: zqT[c, r] ----
            oh = work.tile([128, 4, RBLK], gat_dt, tag="oh")
            for k in range(4):
                nc.vector.tensor_scalar(oh[:, k, :], in0=ixb, scalar1=iotac[:, k:k + 1],
                                        scalar2=None, op0=ALU.is_equal)
            zqsb = work.tile([128, 2, RBLK], dec_dt, tag="zq")
            for m in range(2):
                ps = pp.tile([128, RBLK], F32, tag="ps")
                for k in range(4):
                    _mm(nc, ps, cbs[:, k, m * 128:(m + 1) * 128], oh[:, k, :],
                        gat_dt, k == 0, k == 3)
                nc.vector.tensor_copy(zqsb[:, m, :], ps)

            # ---- decoder layer 1: hT[h, r] ----
            hsb = work.tile([128, 4, RBLK], dec_dt, tag="h")
            for m in range(4):
                ps = pp.tile([128, RBLK], F32, tag="ps")
                for k in range(2):
                    _mm(nc, ps, w3s[:, k, m * 128:(m + 1) * 128], zqsb[:, k, :],
                        dec_dt, k == 0, k == 1)
                nc.scalar.activation(hsb[:, m, :], ps, AF.Relu, bias=b3c[:, m:m + 1])

            # ---- decoder layer 2 (row-major out) + store ----
            for i in range(4):
                psa = pp.tile([128, 512], F32, tag="ps")
                psb = pp.tile([128, 256], F32, tag="ps")
                for k in range(4):
                    _mm(nc, psa, hsb[:, k, i * 128:(i + 1) * 128], w4s[:, k, 0:512],
                        dec_dt, k == 0, k == 3)
                for k in range(4):
                    _mm(nc, psb, hsb[:, k, i * 128:(i + 1) * 128], w4s[:, k, 512:768],
                        dec_dt, k == 0, k == 3)
                rsb = rout.tile([128, D_IN], F32, tag="rsb")
                nc.vector.tensor_add(rsb[:, 0:512], psa, b4b[:, 0:512])
                nc.vector.tensor_add(rsb[:, 512:768], psb, b4b[:, 512:768])
                nc.sync.dma_start(recon.ap()[r0 + i * 128:r0 + (i + 1) * 128, :], rsb)

        nc.sync.dma_start(lz2_o.ap(), accz2)
        nc.sync.dma_start(lmx_o.ap(), accmx)

    nc.compile()
    return nc


_CACHE = {}


def _get_nc(key=("f32", "f32r", "f32r"), rows=ROWS):
    dtmap = {"f32": F32, "f32r": mybir.dt.float32r}
    k = (key, rows)
    if k not in _CACHE:
        _CACHE[k] = build_nc(rows=rows, enc_dt=dtmap[key[0]], gat_dt=dtmap[key[1]],
                             dec_dt=dtmap[key[2]])
    return _CACHE[k]


def make_in_maps(x, W1, b1, W2, b2, codebook, W3, b3, W4, b4, rows=ROWS,
                 ncores=NCORES):
    f = np.float32
    shared = {
        "W1": np.ascontiguousarray(W1, f),
        "W2": np.ascontiguousarray(W2, f),
        "CB": np.ascontiguousarray(codebook, f),
        "CBT": np.ascontiguousarray(codebook.T, f),
        "C2H": (np.sum(codebook.astype(np.float64) ** 2, axis=1) / 2.0).astype(f),
        "W3": np.ascontiguousarray(W3, f),
        "W4": np.ascontiguousarray(W4, f),
        "B1": np.ascontiguousarray(b1.reshape(4, 128).T, f),
        "B2": np.ascontiguousarray(b2.reshape(2, 128).T, f),
        "B3": np.ascontiguousarray(b3.reshape(4, 128).T, f),
        "B4": np.ascontiguousarray(b4, f),
        "IOTA": np.ascontiguousarray(
            (np.arange(128)[:, None] + 128 * np.arange(4)[None, :]), f),
    }
    in_maps = []
    for c in range(ncores):
        xs = x[c * rows:(c + 1) * rows]
        m = dict(shared)
        m["xT"] = np.ascontiguousarray(xs.T.astype(f))
        in_maps.append(m)
    return in_maps


def assemble(results, rows=ROWS):
    recon = np.concatenate([r["recon"] for r in results], axis=0)
    idx = np.concatenate([r["idx"] for r in results], axis=0).astype(np.int32)
    s = 0.0
    for r in results:
        s += np.sum(r["loss_z2"].astype(np.float64))
        s -= 2.0 * np.sum(r["loss_mx"].astype(np.float64))
    loss = np.float32(s / (len(results) * rows * D_CODE))
    return recon, idx, loss


def kernel(**inputs):
    nc = _get_nc()
    in_maps = make_in_maps(**inputs)
    res = run_bass_kernel_spmd(nc, in_maps, core_ids=list(range(NCORES)))
    return assemble(res.results)


if __name__ == "__main__":
    ins = {k: np.asarray(v) for k, v in np.load("inputs.npz").items()}
    out = kernel(**ins)
    print(out[0].shape, out[1].shape, out[2])


# revision 8
# speedup vs baseline: 1.0356x; 1.0356x over previous
"""VQ-VAE forward kernel for Trainium2 (Bass/Tile), data-parallel over 8 NeuronCores.

Pipeline per core (rows sharded 8-way, ROWS=16384 rows/core, blocks of R=512 rows):
  encoder    y1T = relu(W1.T @ xT + b1)      [512, R]  (activations kept transposed)
             zT  = W2.T @ y1T + b2           [256, R]
  vq         dots[i] = z @ cbT  (row-major, 128-row chunks), negd = dots - ||c||^2/2
             argmin dist == argmax negd  ->  max / max_index  (top-8 DVE ops)
  gather     one-hot(idx) matmul against codebook -> zqT [256, R]
  decoder    hT = relu(W3.T @ zqT + b3);  recon[i] = hT.T @ W4 + b4 (row-major out)
  loss       sum(z^2) - 2*sum(max0) partials, reduced on host.

Matmul dtype is float32r (full PE rate with fp32 storage) by default, selectable
per stage group below.
"""

import numpy as np

import concourse.bass as bass
from concourse import bacc
import concourse.tile as tile
import concourse.mybir as mybir
from concourse.bass_utils import run_bass_kernel_spmd
from concourse.masks import make_identity
from concourse import library_config

N, D_IN, D_HID, D_CODE, K = 131072, 768, 512, 256, 512
NCORES = 8
ROWS = N // NCORES          # rows per core
RBLK = 512                  # rows per block
F32 = mybir.dt.float32
U32 = mybir.dt.uint32
I32 = mybir.dt.int32
AF = mybir.ActivationFunctionType
ALU = mybir.AluOpType


def _mm(nc, out, lhsT, rhs, dt, start, stop):
    nc.tensor.matmul(out, lhsT, rhs, start=start, stop=stop)


def build_nc(rows=ROWS, enc_dt=F32, gat_dt=mybir.dt.float32r,
             dec_dt=mybir.dt.float32r):
    nblk = rows // RBLK
    nc = bacc.Bacc("TRN2", target_bir_lowering=False, debug=False)

    # inputs (per-core shard of xT; weights replicated)
    xT = nc.dram_tensor("xT", [D_IN, rows], enc_dt, kind="ExternalInput")
    W1 = nc.dram_tensor("W1", [D_IN, D_HID], enc_dt, kind="ExternalInput")
    W2 = nc.dram_tensor("W2", [D_HID, D_CODE], enc_dt, kind="ExternalInput")
    CB = nc.dram_tensor("CB", [K, D_CODE], gat_dt, kind="ExternalInput")
    CBT = nc.dram_tensor("CBT", [D_CODE, K], enc_dt, kind="ExternalInput")
    C2H = nc.dram_tensor("C2H", [K], F32, kind="ExternalInput")
    W3 = nc.dram_tensor("W3", [D_CODE, D_HID], dec_dt, kind="ExternalInput")
    W4 = nc.dram_tensor("W4", [D_HID, D_IN], dec_dt, kind="ExternalInput")
    B1 = nc.dram_tensor("B1", [128, 4], F32, kind="ExternalInput")   # b1[128c+p] at [p,c]
    B2 = nc.dram_tensor("B2", [128, 2], F32, kind="ExternalInput")
    B3 = nc.dram_tensor("B3", [128, 4], F32, kind="ExternalInput")
    B4 = nc.dram_tensor("B4", [D_IN], F32, kind="ExternalInput")
    IOTA = nc.dram_tensor("IOTA", [128, 4], F32, kind="ExternalInput")  # 128c+p at [p,c]

    recon = nc.dram_tensor("recon", [rows, D_IN], F32, kind="ExternalOutput")
    idx_o = nc.dram_tensor("idx", [rows], I32, kind="ExternalOutput")
    lz2_o = nc.dram_tensor("loss_z2", [128, 2 * nblk], F32, kind="ExternalOutput")
    lmx_o = nc.dram_tensor("loss_mx", [128, 4 * nblk], F32, kind="ExternalOutput")

    with tile.TileContext(nc) as tc, tc.tile_pool(name="consts", bufs=1) as consts, \
            tc.tile_pool(name="work", bufs=2) as work, \
            tc.tile_pool(name="small", bufs=4) as small, \
            tc.tile_pool(name="rout", bufs=4) as rout, \
            tc.tile_pool(name="psum", bufs=7, space="PSUM") as pp:
        # ---- constants into SBUF ----
        w1s = consts.tile([128, 6, D_HID], enc_dt)
        nc.sync.dma_start(w1s, W1.ap().rearrange("(c p) n -> p c n", p=128))
        w2s = consts.tile([128, 4, D_CODE], enc_dt)
        nc.sync.dma_start(w2s, W2.ap().rearrange("(c p) n -> p c n", p=128))
        cbs = consts.tile([128, 4, D_CODE], gat_dt)
        nc.sync.dma_start(cbs, CB.ap().rearrange("(c p) n -> p c n", p=128))
        cbts = consts.tile([128, 2, K], enc_dt)
        nc.sync.dma_start(cbts, CBT.ap().rearrange("(c p) n -> p c n", p=128))
        w3s = consts.tile([128, 2, D_HID], dec_dt)
        nc.sync.dma_start(w3s, W3.ap().rearrange("(c p) n -> p c n", p=128))
        w4s = consts.tile([128, 4, D_IN], dec_dt)
        nc.sync.dma_start(w4s, W4.ap().rearrange("(c p) n -> p c n", p=128))
        c2hb = consts.tile([128, K], F32)
        nc.sync.dma_start(c2hb, bass.AP(tensor=C2H, offset=0, ap=[[0, 128], [1, K]]))
        b4b = consts.tile([128, D_IN], F32)
        nc.sync.dma_start(b4b, bass.AP(tensor=B4, offset=0, ap=[[0, 128], [1, D_IN]]))
        b1c = consts.tile([128, 4], F32)
        nc.sync.dma_start(b1c, B1.ap())
        b2c = consts.tile([128, 2], F32)
        nc.sync.dma_start(b2c, B2.ap())
        b3c = consts.tile([128, 4], F32)
        nc.sync.dma_start(b3c, B3.ap())
        iotac = consts.tile([128, 4], F32)
        nc.sync.dma_start(iotac, IOTA.ap())
        ident = consts.tile([128, 128], F32)
        make_identity(nc, ident[:])
        nc.gpsimd.load_library(library_config.mlp)

        accz2 = consts.tile([128, 2 * nblk], F32)
        accmx = consts.tile([128, 4 * nblk], F32)

        xT_r = xT.ap().rearrange("(c p) r -> p c r", p=128)

        def emit_front(b):
            r0 = b * RBLK
            xt = work.tile([128, 6, RBLK], enc_dt, tag="xt")
            nc.sync.dma_start(xt, xT_r[:, :, r0:r0 + RBLK])

            # ---- encoder layer 1: y1T[h, r] ----
            y1sb = work.tile([128, 4, RBLK], enc_dt, tag="y1")
            for m in range(4):
                ps = pp.tile([128, RBLK], F32, tag="ps")
                for k in range(6):
                    _mm(nc, ps, w1s[:, k, m * 128:(m + 1) * 128], xt[:, k, :],
                        enc_dt, k == 0, k == 5)
                nc.scalar.activation(y1sb[:, m, :], ps, AF.Relu, bias=b1c[:, m:m + 1])

            # ---- encoder layer 2: zT[c, r] ----
            zsb = work.tile([128, 2, RBLK], enc_dt, tag="z")
            for m in range(2):
                ps = pp.tile([128, RBLK], F32, tag="ps")
                for k in range(4):
                    _mm(nc, ps, w2s[:, k, m * 128:(m + 1) * 128], y1sb[:, k, :],
                        enc_dt, k == 0, k == 3)
                nc.scalar.activation(zsb[:, m, :], ps, AF.Identity, bias=b2c[:, m:m + 1])
                zsq = work.tile([128, RBLK], F32, tag="zsq")
                nc.scalar.activation(zsq, zsb[:, m, :], AF.Square,
                                     accum_out=accz2[:, 2 * b + m:2 * b + m + 1])

            # ---- distances + argmin per 128-row chunk ----
            ixf = small.tile([128, 4], F32, tag="ixf")
            for i in range(4):
                ps = pp.tile([128, K], F32, tag="ps")
                _mm(nc, ps, zsb[:, 0, i * 128:(i + 1) * 128], cbts[:, 0, :],
                    enc_dt, True, False)
                _mm(nc, ps, zsb[:, 1, i * 128:(i + 1) * 128], cbts[:, 1, :],
                    enc_dt, False, True)
                negd = work.tile([128, K], F32, tag="negd")
                nc.vector.scalar_tensor_tensor(negd, in0=ps, scalar=1.0, in1=c2hb,
                                               op0=ALU.mult, op1=ALU.subtract)
                mx = small.tile([128, 8], F32, tag="mx")
                nc.vector.max(mx, negd)
                ix = small.tile([128, 8], U32, tag="ix")
                nc.vector.max_index(ix, mx, negd)
                nc.scalar.copy(accmx[:, 4 * b + i:4 * b + i + 1], mx[:, 0:1])
                nc.gpsimd.tensor_copy(ixf[:, i:i + 1], ix[:, 0:1])
            return ixf

        def emit_back(b, ixf):
            r0 = b * RBLK
            # ---- transpose indices to [1, RBLK], write out, broadcast ----
            tps = pp.tile([1, RBLK], F32, tag="ps")
            for i in range(4):
                nc.tensor.matmul(tps[:, i * 128:(i + 1) * 128], ixf[:, i:i + 1],
                                 ident, is_transpose=True, start=(i == 0), stop=(i == 3))
            ixT1 = small.tile([1, RBLK], F32, tag="ixT")
            nc.vector.tensor_copy(ixT1, tps)
            ixi = small.tile([1, RBLK], I32, tag="ixi")
            nc.vector.tensor_copy(ixi, ixT1)
            nc.sync.dma_start(idx_o.ap()[r0:r0 + RBLK].unsqueeze(0), ixi)
            ixb = work.tile([128, RBLK], F32, tag="ixb")
            nc.gpsimd.partition_broadcast(ixb, ixT1, channels=128)

            # ---- one-hot + gather: zqT[c, r] ----
            oh = work.tile([128, 4, RBLK], gat_dt, tag="oh")
            for k in range(4):
                nc.vector.tensor_scalar(oh[:, k, :], in0=ixb, scalar1=iotac[:, k:k + 1],
                                        scalar2=None, op0=ALU.is_equal)
            zqsb = work.tile([128, 2, RBLK], dec_dt, tag="zq")
            for m in range(2):
                ps = pp.tile([128, RBLK], F32, tag="ps")
                for k in range(4):
                    _mm(nc, ps, cbs[:, k, m * 128:(m + 1) * 128], oh[:, k, :],
                        gat_dt, k == 0, k == 3)
                nc.vector.tensor_copy(zqsb[:, m, :], ps)

            # ---- decoder layer 1: hT[h, r] ----
            hsb = work.tile([128, 4, RBLK], dec_dt, tag="h")
            for m in range(4):
                ps = pp.tile([128, RBLK], F32, tag="ps")
                for k in range(2):
                    _mm(nc, ps, w3s[:, k, m * 128:(m + 1) * 128], zqsb[:, k, :],
                        dec_dt, k == 0, k == 1)
                nc.scalar.activation(hsb[:, m, :], ps, AF.Relu, bias=b3c[:, m:m + 1])

            # ---- decoder layer 2 (row-major out) + store ----
            for i in range(4):
                psa = pp.tile([128, 512], F32, tag="ps")
                psb = pp.tile([128, 256], F32, tag="ps")
                for k in range(4):
                    _mm(nc, psa, hsb[:, k, i * 128:(i + 1) * 128], w4s[:, k, 0:512],
                        dec_dt, k == 0, k == 3)
                for k in range(4):
                    _mm(nc, psb, hsb[:, k, i * 128:(i + 1) * 128], w4s[:, k, 512:768],
                        dec_dt, k == 0, k == 3)
                rsb = rout.tile([128, D_IN], F32, tag="rsb")
                nc.vector.tensor_add(rsb[:, 0:512], psa, b4b[:, 0:512])
                nc.vector.tensor_add(rsb[:, 512:768], psb, b4b[:, 512:768])
                nc.sync.dma_start(recon.ap()[r0 + i * 128:r0 + (i + 1) * 128, :], rsb)

        # software-pipelined emission: encoder(b) ahead of decoder(b-1) so the
        # PE never stalls on the DVE argmin chain (keeps HAM warm too)
        prev = None
        for b in range(nblk):
            ixf = emit_front(b)
            if prev is not None:
                emit_back(b - 1, prev)
            prev = ixf
        emit_back(nblk - 1, prev)

        nc.sync.dma_start(lz2_o.ap(), accz2)
        nc.sync.dma_start(lmx_o.ap(), accmx)

    nc.compile()
    return nc


_CACHE = {}


def _get_nc(key=("f32", "f32r", "f32r"), rows=ROWS):
    dtmap = {"f32": F32, "f32r": mybir.dt.float32r}
    k = (key, rows)
    if k not in _CACHE:
        _CACHE[k] = build_nc(rows=rows, enc_dt=dtmap[key[0]], gat_dt=dtmap[key[1]],
                             dec_dt=dtmap[key[2]])
    return _CACHE[k]


def make_in_maps(x, W1, b1, W2, b2, codebook, W3, b3, W4, b4, rows=ROWS,
                 ncores=NCORES):
    f = np.float32
    shared = {
        "W1": np.ascontiguousarray(W1, f),
        "W2": np.ascontiguousarray(W2, f),
        "CB": np.ascontiguousarray(codebook, f),
        "CBT": np.ascontiguousarray(codebook.T, f),
        "C2H": (np.sum(codebook.astype(np.float64) ** 2, axis=1) / 2.0).astype(f),
        "W3": np.ascontiguousarray(W3, f),
        "W4": np.ascontiguousarray(W4, f),
        "B1": np.ascontiguousarray(b1.reshape(4, 128).T, f),
        "B2": np.ascontiguousarray(b2.reshape(2, 128).T, f),
        "B3": np.ascontiguousarray(b3.reshape(4, 128).T, f),
        "B4": np.ascontiguousarray(b4, f),
        "IOTA": np.ascontiguousarray(
            (np.arange(128)[:, None] + 128 * np.arange(4)[None, :]), f),
    }
    in_maps = []
    for c in range(ncores):
        xs = x[c * rows:(c + 1) * rows]
        m = dict(shared)
        m["xT"] = np.ascontiguousarray(xs.T.astype(f))
        in_maps.append(m)
    return in_maps


def assemble(results, rows=ROWS):
    recon = np.concatenate([r["recon"] for r in results], axis=0)
    idx = np.concatenate([r["idx"] for r in results], axis=0).astype(np.int32)
    s = 0.0
    for r in results:
        s += np.sum(r["loss_z2"].astype(np.float64))
        s -= 2.0 * np.sum(r["loss_mx"].astype(np.float64))
    loss = np.float32(s / (len(results) * rows * D_CODE))
    return recon, idx, loss


def kernel(**inputs):
    nc = _get_nc()
    in_maps = make_in_maps(**inputs)
    res = run_bass_kernel_spmd(nc, in_maps, core_ids=list(range(NCORES)))
    return assemble(res.results)


if __name__ == "__main__":
    ins = {k: np.asarray(v) for k, v in np.load("inputs.npz").items()}
    out = kernel(**ins)
    print(out[0].shape, out[1].shape, out[2])


# revision 10
# speedup vs baseline: 1.0422x; 1.0064x over previous
"""VQ-VAE forward kernel for Trainium2 (Bass/Tile), data-parallel over 8 NeuronCores.

Pipeline per core (rows sharded 8-way, ROWS=16384 rows/core, blocks of R=512 rows):
  encoder    y1T = relu(W1.T @ xT + b1)      [512, R]  (activations kept transposed)
             zT  = W2.T @ y1T + b2           [256, R]
  vq         dots[i] = z @ cbT  (row-major, 128-row chunks), negd = dots - ||c||^2/2
             argmin dist == argmax negd  ->  max / max_index  (top-8 DVE ops)
  gather     one-hot(idx) matmul against codebook -> zqT [256, R]
  decoder    hT = relu(W3.T @ zqT + b3);  recon[i] = hT.T @ W4 + b4 (row-major out)
  loss       sum(z^2) - 2*sum(max0) partials, reduced on host.

Matmul dtype is float32r (full PE rate with fp32 storage) by default, selectable
per stage group below.
"""

import numpy as np

import concourse.bass as bass
from concourse import bacc
import concourse.tile as tile
import concourse.mybir as mybir
from concourse.bass_utils import run_bass_kernel_spmd
from concourse.masks import make_identity
from concourse import library_config

N, D_IN, D_HID, D_CODE, K = 131072, 768, 512, 256, 512
NCORES = 8
ROWS = N // NCORES          # rows per core
RBLK = 512                  # rows per block
F32 = mybir.dt.float32
U32 = mybir.dt.uint32
I32 = mybir.dt.int32
AF = mybir.ActivationFunctionType
ALU = mybir.AluOpType


def _mm(nc, out, lhsT, rhs, dt, start, stop):
    nc.tensor.matmul(out, lhsT, rhs, start=start, stop=stop)


def build_nc(rows=ROWS, enc_dt=F32, gat_dt=mybir.dt.float32r,
             dec_dt=mybir.dt.float32r):
    nblk = rows // RBLK
    nc = bacc.Bacc("TRN2", target_bir_lowering=False, debug=False)

    # inputs (per-core shard of xT; weights replicated)
    xT = nc.dram_tensor("xT", [D_IN, rows], enc_dt, kind="ExternalInput")
    W1 = nc.dram_tensor("W1", [D_IN, D_HID], enc_dt, kind="ExternalInput")
    W2 = nc.dram_tensor("W2", [D_HID, D_CODE], enc_dt, kind="ExternalInput")
    CB = nc.dram_tensor("CB", [K, D_CODE], gat_dt, kind="ExternalInput")
    CBT = nc.dram_tensor("CBT", [D_CODE, K], enc_dt, kind="ExternalInput")
    C2H = nc.dram_tensor("C2H", [K], F32, kind="ExternalInput")
    W3 = nc.dram_tensor("W3", [D_CODE, D_HID], dec_dt, kind="ExternalInput")
    W4 = nc.dram_tensor("W4", [D_HID, D_IN], dec_dt, kind="ExternalInput")
    B1 = nc.dram_tensor("B1", [128, 4], F32, kind="ExternalInput")   # b1[128c+p] at [p,c]
    B2 = nc.dram_tensor("B2", [128, 2], F32, kind="ExternalInput")
    B3 = nc.dram_tensor("B3", [128, 4], F32, kind="ExternalInput")
    B4 = nc.dram_tensor("B4", [D_IN], F32, kind="ExternalInput")
    IOTA = nc.dram_tensor("IOTA", [128, 4], F32, kind="ExternalInput")  # 128c+p at [p,c]

    recon = nc.dram_tensor("recon", [rows, D_IN], F32, kind="ExternalOutput")
    idx_o = nc.dram_tensor("idx", [rows], I32, kind="ExternalOutput")
    lz2_o = nc.dram_tensor("loss_z2", [128, 2 * nblk], F32, kind="ExternalOutput")
    lmx_o = nc.dram_tensor("loss_mx", [128, 4 * nblk], F32, kind="ExternalOutput")

    with tile.TileContext(nc) as tc, tc.tile_pool(name="consts", bufs=1) as consts, \
            tc.tile_pool(name="work", bufs=2) as work, \
            tc.tile_pool(name="small", bufs=4) as small, \
            tc.tile_pool(name="rout", bufs=4) as rout, \
            tc.tile_pool(name="psum", bufs=7, space="PSUM") as pp:
        # ---- constants into SBUF ----
        w1s = consts.tile([128, 6, D_HID], enc_dt)
        nc.sync.dma_start(w1s, W1.ap().rearrange("(c p) n -> p c n", p=128))
        w2s = consts.tile([128, 4, D_CODE], enc_dt)
        nc.sync.dma_start(w2s, W2.ap().rearrange("(c p) n -> p c n", p=128))
        cbs = consts.tile([128, 4, D_CODE], gat_dt)
        nc.sync.dma_start(cbs, CB.ap().rearrange("(c p) n -> p c n", p=128))
        cbts = consts.tile([128, 2, K], enc_dt)
        nc.sync.dma_start(cbts, CBT.ap().rearrange("(c p) n -> p c n", p=128))
        w3s = consts.tile([128, 2, D_HID], dec_dt)
        nc.sync.dma_start(w3s, W3.ap().rearrange("(c p) n -> p c n", p=128))
        w4s = consts.tile([128, 4, D_IN], dec_dt)
        nc.sync.dma_start(w4s, W4.ap().rearrange("(c p) n -> p c n", p=128))
        c2hb = consts.tile([128, K], F32)
        nc.sync.dma_start(c2hb, bass.AP(tensor=C2H, offset=0, ap=[[0, 128], [1, K]]))
        b4b = consts.tile([128, D_IN], F32)
        nc.sync.dma_start(b4b, bass.AP(tensor=B4, offset=0, ap=[[0, 128], [1, D_IN]]))
        b1c = consts.tile([128, 4], F32)
        nc.sync.dma_start(b1c, B1.ap())
        b2c = consts.tile([128, 2], F32)
        nc.sync.dma_start(b2c, B2.ap())
        b3c = consts.tile([128, 4], F32)
        nc.sync.dma_start(b3c, B3.ap())
        iotac = consts.tile([128, 4], F32)
        nc.sync.dma_start(iotac, IOTA.ap())
        ident = consts.tile([128, 128], F32)
        make_identity(nc, ident[:])
        nc.gpsimd.load_library(library_config.mlp)

        accz2 = consts.tile([128, 2 * nblk], F32)
        accmx = consts.tile([128, 4 * nblk], F32)

        xT_r = xT.ap().rearrange("(c p) r -> p c r", p=128)

        def emit_front(b):
            r0 = b * RBLK
            xt = work.tile([128, 6, RBLK], enc_dt, tag="xt")
            nc.sync.dma_start(xt, xT_r[:, :, r0:r0 + RBLK])

            # ---- encoder layer 1: y1T[h, r] ----
            y1sb = work.tile([128, 4, RBLK], enc_dt, tag="y1")
            for m in range(4):
                ps = pp.tile([128, RBLK], F32, tag="ps")
                for k in range(6):
                    _mm(nc, ps, w1s[:, k, m * 128:(m + 1) * 128], xt[:, k, :],
                        enc_dt, k == 0, k == 5)
                nc.scalar.activation(y1sb[:, m, :], ps, AF.Relu, bias=b1c[:, m:m + 1])

            # ---- encoder layer 2: zT[c, r] ----
            zsb = work.tile([128, 2, RBLK], enc_dt, tag="z")
            for m in range(2):
                ps = pp.tile([128, RBLK], F32, tag="ps")
                for k in range(4):
                    _mm(nc, ps, w2s[:, k, m * 128:(m + 1) * 128], y1sb[:, k, :],
                        enc_dt, k == 0, k == 3)
                nc.scalar.activation(zsb[:, m, :], ps, AF.Identity, bias=b2c[:, m:m + 1])
                zsq = work.tile([128, RBLK], F32, tag="zsq")
                nc.scalar.activation(zsq, zsb[:, m, :], AF.Square,
                                     accum_out=accz2[:, 2 * b + m:2 * b + m + 1])

            # ---- distances + argmin per 128-row chunk ----
            ixf = small.tile([128, 4], F32, tag="ixf")
            for i in range(4):
                ps = pp.tile([128, K], F32, tag="ps")
                _mm(nc, ps, zsb[:, 0, i * 128:(i + 1) * 128], cbts[:, 0, :],
                    enc_dt, True, False)
                _mm(nc, ps, zsb[:, 1, i * 128:(i + 1) * 128], cbts[:, 1, :],
                    enc_dt, False, True)
                negd = work.tile([128, K], F32, tag="negd")
                nc.vector.scalar_tensor_tensor(negd, in0=ps, scalar=1.0, in1=c2hb,
                                               op0=ALU.mult, op1=ALU.subtract)
                mx = small.tile([128, 8], F32, tag="mx")
                nc.vector.max(mx, negd)
                ix = small.tile([128, 8], U32, tag="ix")
                nc.vector.max_index(ix, mx, negd)
                nc.scalar.copy(accmx[:, 4 * b + i:4 * b + i + 1], mx[:, 0:1])
                nc.gpsimd.tensor_copy(ixf[:, i:i + 1], ix[:, 0:1])
            return ixf

        def emit_mid(b, ixf):
            r0 = b * RBLK
            # ---- transpose indices to [1, RBLK], write out, broadcast ----
            tps = pp.tile([1, RBLK], F32, tag="ps")
            for i in range(4):
                nc.tensor.matmul(tps[:, i * 128:(i + 1) * 128], ixf[:, i:i + 1],
                                 ident, is_transpose=True, start=(i == 0), stop=(i == 3))
            ixT1 = small.tile([1, RBLK], F32, tag="ixT")
            nc.vector.tensor_copy(ixT1, tps)
            ixi = small.tile([1, RBLK], I32, tag="ixi")
            nc.vector.tensor_copy(ixi, ixT1)
            nc.sync.dma_start(idx_o.ap()[r0:r0 + RBLK].unsqueeze(0), ixi)
            ixb = work.tile([128, RBLK], F32, tag="ixb")
            nc.gpsimd.partition_broadcast(ixb, ixT1, channels=128)

            # ---- one-hot ----
            oh = work.tile([128, 4, RBLK], gat_dt, tag="oh", bufs=3)
            for k in range(4):
                nc.vector.tensor_scalar(oh[:, k, :], in0=ixb, scalar1=iotac[:, k:k + 1],
                                        scalar2=None, op0=ALU.is_equal)
            return oh

        def emit_back(b, oh):
            r0 = b * RBLK
            # ---- gather: zqT[c, r] ----
            zqsb = work.tile([128, 2, RBLK], dec_dt, tag="zq")
            for m in range(2):
                ps = pp.tile([128, RBLK], F32, tag="ps")
                for k in range(4):
                    _mm(nc, ps, cbs[:, k, m * 128:(m + 1) * 128], oh[:, k, :],
                        gat_dt, k == 0, k == 3)
                nc.vector.tensor_copy(zqsb[:, m, :], ps)

            # ---- decoder layer 1: hT[h, r] ----
            hsb = work.tile([128, 4, RBLK], dec_dt, tag="h")
            for m in range(4):
                ps = pp.tile([128, RBLK], F32, tag="ps")
                for k in range(2):
                    _mm(nc, ps, w3s[:, k, m * 128:(m + 1) * 128], zqsb[:, k, :],
                        dec_dt, k == 0, k == 1)
                nc.scalar.activation(hsb[:, m, :], ps, AF.Relu, bias=b3c[:, m:m + 1])

            # ---- decoder layer 2 (row-major out) + store ----
            for i in range(4):
                psa = pp.tile([128, 512], F32, tag="ps")
                psb = pp.tile([128, 256], F32, tag="ps")
                for k in range(4):
                    _mm(nc, psa, hsb[:, k, i * 128:(i + 1) * 128], w4s[:, k, 0:512],
                        dec_dt, k == 0, k == 3)
                for k in range(4):
                    _mm(nc, psb, hsb[:, k, i * 128:(i + 1) * 128], w4s[:, k, 512:768],
                        dec_dt, k == 0, k == 3)
                rsb = rout.tile([128, D_IN], F32, tag="rsb")
                nc.vector.tensor_add(rsb[:, 0:512], psa, b4b[:, 0:512])
                nc.vector.tensor_add(rsb[:, 512:768], psb, b4b[:, 512:768])
                nc.sync.dma_start(recon.ap()[r0 + i * 128:r0 + (i + 1) * 128, :], rsb)

        # 3-stage software-pipelined emission: encoder(b) / transpose+onehot(b-1)
        # / gather+decoder(b-2), so the PE never waits on the DVE argmin or
        # broadcast/one-hot chains (keeps HAM warm too)
        ixfs = {}
        ohs = {}
        for b in range(nblk + 2):
            if b < nblk:
                ixfs[b] = emit_front(b)
            if 1 <= b <= nblk:
                ohs[b - 1] = emit_mid(b - 1, ixfs.pop(b - 1))
            if b >= 2:
                emit_back(b - 2, ohs.pop(b - 2))

        nc.sync.dma_start(lz2_o.ap(), accz2)
        nc.sync.dma_start(lmx_o.ap(), accmx)

    nc.compile()
    return nc


_CACHE = {}


def _get_nc(key=("f32", "f32r", "f32r"), rows=ROWS):
    dtmap = {"f32": F32, "f32r": mybir.dt.float32r}
    k = (key, rows)
    if k not in _CACHE:
        _CACHE[k] = build_nc(rows=rows, enc_dt=dtmap[key[0]], gat_dt=dtmap[key[1]],
                             dec_dt=dtmap[key[2]])
    return _CACHE[k]


def make_in_maps(x, W1, b1, W2, b2, codebook, W3, b3, W4, b4, rows=ROWS,
                 ncores=NCORES):
    f = np.float32
    shared = {
        "W1": np.ascontiguousarray(W1, f),
        "W2": np.ascontiguousarray(W2, f),
        "CB": np.ascontiguousarray(codebook, f),
        "CBT": np.ascontiguousarray(codebook.T, f),
        "C2H": (np.sum(codebook.astype(np.float64) ** 2, axis=1) / 2.0).astype(f),
        "W3": np.ascontiguousarray(W3, f),
        "W4": np.ascontiguousarray(W4, f),
        "B1": np.ascontiguousarray(b1.reshape(4, 128).T, f),
        "B2": np.ascontiguousarray(b2.reshape(2, 128).T, f),
        "B3": np.ascontiguousarray(b3.reshape(4, 128).T, f),
        "B4": np.ascontiguousarray(b4, f),
        "IOTA": np.ascontiguousarray(
            (np.arange(128)[:, None] + 128 * np.arange(4)[None, :]), f),
    }
    in_maps = []
    for c in range(ncores):
        xs = x[c * rows:(c + 1) * rows]
        m = dict(shared)
        m["xT"] = np.ascontiguousarray(xs.T.astype(f))
        in_maps.append(m)
    return in_maps


def assemble(results, rows=ROWS):
    recon = np.concatenate([r["recon"] for r in results], axis=0)
    idx = np.concatenate([r["idx"] for r in results], axis=0).astype(np.int32)
    s = 0.0
    for r in results:
        s += np.sum(r["loss_z2"].astype(np.float64))
        s -= 2.0 * np.sum(r["loss_mx"].astype(np.float64))
    loss = np.float32(s / (len(results) * rows * D_CODE))
    return recon, idx, loss


def kernel(**inputs):
    nc = _get_nc()
    in_maps = make_in_maps(**inputs)
    res = run_bass_kernel_spmd(nc, in_maps, core_ids=list(range(NCORES)))
    return assemble(res.results)


if __name__ == "__main__":
    ins = {k: np.asarray(v) for k, v in np.load("inputs.npz").items()}
    out = kernel(**ins)
    print(out[0].shape, out[1].shape, out[2])


# revision 11
# speedup vs baseline: 1.1599x; 1.1129x over previous
"""VQ-VAE forward kernel for Trainium2 (Bass/Tile), data-parallel over 8 NeuronCores.

Pipeline per core (rows sharded 8-way, ROWS=16384 rows/core, blocks of R=512 rows):
  encoder    y1T = relu(W1.T @ xT + b1)      [512, R]  (activations kept transposed)
             zT  = W2.T @ y1T + b2           [256, R]
  vq         dots[i] = z @ cbT  (row-major, 128-row chunks), negd = dots - ||c||^2/2
             argmin dist == argmax negd  ->  max / max_index  (top-8 DVE ops)
  gather     one-hot(idx) matmul against codebook -> zqT [256, R]
  decoder    hT = relu(W3.T @ zqT + b3);  recon[i] = hT.T @ W4 + b4 (row-major out)
  loss       sum(z^2) - 2*sum(max0) partials, reduced on host.

Matmul dtype is float32r (full PE rate with fp32 storage) by default, selectable
per stage group below.
"""

import numpy as np

import concourse.bass as bass
from concourse import bacc
import concourse.tile as tile
import concourse.mybir as mybir
from concourse.bass_utils import run_bass_kernel_spmd
from concourse.masks import make_identity
from concourse import library_config

N, D_IN, D_HID, D_CODE, K = 131072, 768, 512, 256, 512
NCORES = 8
ROWS = N // NCORES          # rows per core
RBLK = 512                  # rows per block
F32 = mybir.dt.float32
U32 = mybir.dt.uint32
I32 = mybir.dt.int32
AF = mybir.ActivationFunctionType
ALU = mybir.AluOpType


def _mm(nc, out, lhsT, rhs, dt, start, stop):
    nc.tensor.matmul(out, lhsT, rhs, start=start, stop=stop)


def build_nc(rows=ROWS, enc_dt=F32, gat_dt=mybir.dt.float32r,
             dec_dt=mybir.dt.float32r):
    nblk = rows // RBLK
    nc = bacc.Bacc("TRN2", target_bir_lowering=False, debug=False)

    # inputs (per-core shard of xT; weights replicated)
    xT = nc.dram_tensor("xT", [D_IN, rows], enc_dt, kind="ExternalInput")
    W1 = nc.dram_tensor("W1", [D_IN, D_HID], enc_dt, kind="ExternalInput")
    W2 = nc.dram_tensor("W2", [D_HID, D_CODE], enc_dt, kind="ExternalInput")
    CB = nc.dram_tensor("CB", [K, D_CODE], gat_dt, kind="ExternalInput")
    CBT = nc.dram_tensor("CBT", [D_CODE, K], enc_dt, kind="ExternalInput")
    C2H = nc.dram_tensor("C2H", [K], F32, kind="ExternalInput")
    W3 = nc.dram_tensor("W3", [D_CODE, D_HID], dec_dt, kind="ExternalInput")
    W4 = nc.dram_tensor("W4", [D_HID, D_IN], dec_dt, kind="ExternalInput")
    B1 = nc.dram_tensor("B1", [128, 4], F32, kind="ExternalInput")   # b1[128c+p] at [p,c]
    B2 = nc.dram_tensor("B2", [128, 2], F32, kind="ExternalInput")
    B3 = nc.dram_tensor("B3", [128, 4], F32, kind="ExternalInput")
    B4 = nc.dram_tensor("B4", [D_IN], F32, kind="ExternalInput")
    IOTA = nc.dram_tensor("IOTA", [128, 4], F32, kind="ExternalInput")  # 128c+p at [p,c]

    recon = nc.dram_tensor("recon", [rows, D_IN], F32, kind="ExternalOutput")
    idx_o = nc.dram_tensor("idx", [rows], I32, kind="ExternalOutput")
    lz2_o = nc.dram_tensor("loss_z2", [128, 2 * nblk], F32, kind="ExternalOutput")
    lmx_o = nc.dram_tensor("loss_mx", [128, 4 * nblk], F32, kind="ExternalOutput")

    with tile.TileContext(nc) as tc, tc.tile_pool(name="consts", bufs=1) as consts, \
            tc.tile_pool(name="work", bufs=2) as work, \
            tc.tile_pool(name="small", bufs=4) as small, \
            tc.tile_pool(name="rout", bufs=4) as rout, \
            tc.tile_pool(name="psum", bufs=7, space="PSUM") as pp:
        # ---- constants into SBUF ----
        w1s = consts.tile([128, 6, D_HID], enc_dt)
        nc.sync.dma_start(w1s, W1.ap().rearrange("(c p) n -> p c n", p=128))
        w2s = consts.tile([128, 4, D_CODE], enc_dt)
        nc.sync.dma_start(w2s, W2.ap().rearrange("(c p) n -> p c n", p=128))
        cbs = consts.tile([128, 4, D_CODE], gat_dt)
        nc.sync.dma_start(cbs, CB.ap().rearrange("(c p) n -> p c n", p=128))
        cbts = consts.tile([128, 2, K], enc_dt)
        nc.sync.dma_start(cbts, CBT.ap().rearrange("(c p) n -> p c n", p=128))
        w3s = consts.tile([128, 2, D_HID], dec_dt)
        nc.sync.dma_start(w3s, W3.ap().rearrange("(c p) n -> p c n", p=128))
        w4s = consts.tile([128, 4, D_IN], dec_dt)
        nc.sync.dma_start(w4s, W4.ap().rearrange("(c p) n -> p c n", p=128))
        c2hb = consts.tile([128, K], F32)
        nc.sync.dma_start(c2hb, bass.AP(tensor=C2H, offset=0, ap=[[0, 128], [1, K]]))
        b4b = consts.tile([128, D_IN], F32)
        nc.sync.dma_start(b4b, bass.AP(tensor=B4, offset=0, ap=[[0, 128], [1, D_IN]]))
        b1c = consts.tile([128, 4], F32)
        nc.sync.dma_start(b1c, B1.ap())
        b2c = consts.tile([128, 2], F32)
        nc.sync.dma_start(b2c, B2.ap())
        b3c = consts.tile([128, 4], F32)
        nc.sync.dma_start(b3c, B3.ap())
        iotac = consts.tile([128, 4], F32)
        nc.sync.dma_start(iotac, IOTA.ap())
        ident = consts.tile([128, 128], F32)
        make_identity(nc, ident[:])
        nc.gpsimd.load_library(library_config.mlp)

        accz2 = consts.tile([128, 2 * nblk], F32)
        accmx = consts.tile([128, 4 * nblk], F32)

        xT_r = xT.ap().rearrange("(c p) r -> p c r", p=128)

        def emit_front(b):
            r0 = b * RBLK
            xt = work.tile([128, 6, RBLK], enc_dt, tag="xt")
            nc.sync.dma_start(xt, xT_r[:, :, r0:r0 + RBLK])

            # ---- encoder layer 1: y1T[h, r] ----
            y1sb = work.tile([128, 4, RBLK], enc_dt, tag="y1")
            for m in range(4):
                ps = pp.tile([128, RBLK], F32, tag="ps")
                for k in range(6):
                    _mm(nc, ps, w1s[:, k, m * 128:(m + 1) * 128], xt[:, k, :],
                        enc_dt, k == 0, k == 5)
                nc.scalar.activation(y1sb[:, m, :], ps, AF.Relu, bias=b1c[:, m:m + 1])

            # ---- encoder layer 2: zT[c, r] ----
            zsb = work.tile([128, 2, RBLK], enc_dt, tag="z")
            for m in range(2):
                ps = pp.tile([128, RBLK], F32, tag="ps")
                for k in range(4):
                    _mm(nc, ps, w2s[:, k, m * 128:(m + 1) * 128], y1sb[:, k, :],
                        enc_dt, k == 0, k == 3)
                nc.scalar.activation(zsb[:, m, :], ps, AF.Identity, bias=b2c[:, m:m + 1])
                zsq = work.tile([128, RBLK], F32, tag="zsq")
                nc.scalar.activation(zsq, zsb[:, m, :], AF.Square,
                                     accum_out=accz2[:, 2 * b + m:2 * b + m + 1])

            # ---- distances + argmin per 128-row chunk ----
            ixf = small.tile([128, 4], F32, tag="ixf")
            for i in range(4):
                ps = pp.tile([128, K], F32, tag="ps")
                _mm(nc, ps, zsb[:, 0, i * 128:(i + 1) * 128], cbts[:, 0, :],
                    enc_dt, True, False)
                _mm(nc, ps, zsb[:, 1, i * 128:(i + 1) * 128], cbts[:, 1, :],
                    enc_dt, False, True)
                negd = work.tile([128, K], F32, tag="negd")
                nc.vector.scalar_tensor_tensor(negd, in0=ps, scalar=1.0, in1=c2hb,
                                               op0=ALU.mult, op1=ALU.subtract)
                mx = small.tile([128, 8], F32, tag="mx")
                nc.vector.max(mx, negd)
                ix = small.tile([128, 8], U32, tag="ix")
                nc.vector.max_index(ix, mx, negd)
                nc.scalar.copy(accmx[:, 4 * b + i:4 * b + i + 1], mx[:, 0:1])
                nc.gpsimd.tensor_copy(ixf[:, i:i + 1], ix[:, 0:1])
            return ixf

        def emit_mid(b, ixf):
            r0 = b * RBLK
            # ---- transpose indices to [1, RBLK], write out, broadcast ----
            tps = pp.tile([1, RBLK], F32, tag="ps")
            for i in range(4):
                nc.tensor.matmul(tps[:, i * 128:(i + 1) * 128], ixf[:, i:i + 1],
                                 ident, is_transpose=True, start=(i == 0), stop=(i == 3))
            ixT1 = small.tile([1, RBLK], F32, tag="ixT")
            nc.vector.tensor_copy(ixT1, tps)
            ixi = small.tile([1, RBLK], I32, tag="ixi")
            nc.vector.tensor_copy(ixi, ixT1)
            nc.sync.dma_start(idx_o.ap()[r0:r0 + RBLK].unsqueeze(0), ixi)
            ixb = work.tile([128, RBLK], F32, tag="ixb")
            nc.gpsimd.partition_broadcast(ixb, ixT1, channels=128)

            # ---- one-hot ----
            oh = work.tile([128, 4, RBLK], gat_dt, tag="oh", bufs=3)
            for k in range(4):
                nc.vector.tensor_scalar(oh[:, k, :], in0=ixb, scalar1=iotac[:, k:k + 1],
                                        scalar2=None, op0=ALU.is_equal)
            return oh

        def emit_back(b, oh):
            r0 = b * RBLK
            # ---- gather: zqT[c, r] ----
            zqsb = work.tile([128, 2, RBLK], dec_dt, tag="zq")
            for m in range(2):
                ps = pp.tile([128, RBLK], F32, tag="ps")
                for k in range(4):
                    _mm(nc, ps, cbs[:, k, m * 128:(m + 1) * 128], oh[:, k, :],
                        gat_dt, k == 0, k == 3)
                with tc.high_priority():
                    nc.vector.tensor_copy(zqsb[:, m, :], ps)

            # ---- decoder layer 1: hT[h, r] ----
            hsb = work.tile([128, 4, RBLK], dec_dt, tag="h")
            for m in range(4):
                ps = pp.tile([128, RBLK], F32, tag="ps")
                for k in range(2):
                    _mm(nc, ps, w3s[:, k, m * 128:(m + 1) * 128], zqsb[:, k, :],
                        dec_dt, k == 0, k == 1)
                with tc.high_priority():
                    nc.scalar.activation(hsb[:, m, :], ps, AF.Relu, bias=b3c[:, m:m + 1])

            # ---- decoder layer 2 (row-major out) + store ----
            for i in range(4):
                psa = pp.tile([128, 512], F32, tag="ps")
                psb = pp.tile([128, 256], F32, tag="ps")
                for k in range(4):
                    _mm(nc, psa, hsb[:, k, i * 128:(i + 1) * 128], w4s[:, k, 0:512],
                        dec_dt, k == 0, k == 3)
                for k in range(4):
                    _mm(nc, psb, hsb[:, k, i * 128:(i + 1) * 128], w4s[:, k, 512:768],
                        dec_dt, k == 0, k == 3)
                rsb = rout.tile([128, D_IN], F32, tag="rsb")
                with tc.high_priority():
                    nc.vector.tensor_add(rsb[:, 0:512], psa, b4b[:, 0:512])
                    nc.vector.tensor_add(rsb[:, 512:768], psb, b4b[:, 512:768])
                nc.sync.dma_start(recon.ap()[r0 + i * 128:r0 + (i + 1) * 128, :], rsb)

        # 3-stage software-pipelined emission: encoder(b) / transpose+onehot(b-1)
        # / gather+decoder(b-2), so the PE never waits on the DVE argmin or
        # broadcast/one-hot chains (keeps HAM warm too)
        ixfs = {}
        ohs = {}
        for b in range(nblk + 2):
            if b < nblk:
                ixfs[b] = emit_front(b)
            if 1 <= b <= nblk:
                ohs[b - 1] = emit_mid(b - 1, ixfs.pop(b - 1))
            if b >= 2:
                emit_back(b - 2, ohs.pop(b - 2))

        nc.sync.dma_start(lz2_o.ap(), accz2)
        nc.sync.dma_start(lmx_o.ap(), accmx)

    nc.compile()
    return nc


_CACHE = {}


def _get_nc(key=("f32", "f32r", "f32r"), rows=ROWS):
    dtmap = {"f32": F32, "f32r": mybir.dt.float32r}
    k = (key, rows)
    if k not in _CACHE:
        _CACHE[k] = build_nc(rows=rows, enc_dt=dtmap[key[0]], gat_dt=dtmap[key[1]],
                             dec_dt=dtmap[key[2]])
    return _CACHE[k]


def make_in_maps(x, W1, b1, W2, b2, codebook, W3, b3, W4, b4, rows=ROWS,
                 ncores=NCORES):
    f = np.float32
    shared = {
        "W1": np.ascontiguousarray(W1, f),
        "W2": np.ascontiguousarray(W2, f),
        "CB": np.ascontiguousarray(codebook, f),
        "CBT": np.ascontiguousarray(codebook.T, f),
        "C2H": (np.sum(codebook.astype(np.float64) ** 2, axis=1) / 2.0).astype(f),
        "W3": np.ascontiguousarray(W3, f),
        "W4": np.ascontiguousarray(W4, f),
        "B1": np.ascontiguousarray(b1.reshape(4, 128).T, f),
        "B2": np.ascontiguousarray(b2.reshape(2, 128).T, f),
        "B3": np.ascontiguousarray(b3.reshape(4, 128).T, f),
        "B4": np.ascontiguousarray(b4, f),
        "IOTA": np.ascontiguousarray(
            (np.arange(128)[:, None] + 128 * np.arange(4)[None, :]), f),
    }
    in_maps = []
    for c in range(ncores):
        xs = x[c * rows:(c + 1) * rows]
        m = dict(shared)
        m["xT"] = np.ascontiguousarray(xs.T.astype(f))
        in_maps.append(m)
    return in_maps


def assemble(results, rows=ROWS):
    recon = np.concatenate([r["recon"] for r in results], axis=0)
    idx = np.concatenate([r["idx"] for r in results], axis=0).astype(np.int32)
    s = 0.0
    for r in results:
        s += np.sum(r["loss_z2"].astype(np.float64))
        s -= 2.0 * np.sum(r["loss_mx"].astype(np.float64))
    loss = np.float32(s / (len(results) * rows * D_CODE))
    return recon, idx, loss


def kernel(**inputs):
    nc = _get_nc()
    in_maps = make_in_maps(**inputs)
    res = run_bass_kernel_spmd(nc, in_maps, core_ids=list(range(NCORES)))
    return assemble(res.results)


if __name__ == "__main__":
    ins = {k: np.asarray(v) for k, v in np.load("inputs.npz").items()}
    out = kernel(**ins)
    print(out[0].shape, out[1].shape, out[2])
